# revision 2
# baseline (speedup 1.0000x reference)
"""Trainium2 Bass kernel for nn_LogReg_30193620091430.

Pipeline per core (data-parallel over 8 cores, 1250 graphs / 125k nodes each):
  stage 1: segment-sum of fixed-length (100-node) graphs via f32r matmuls
           with precomputed 0/1 segment patterns as the stationary operand.
           Nodes stream in natural layout [128 nodes, 256 feat]; PSUM
           accumulates 32-graph groups (25 node-tiles each).
  stage 2: per 32-graph group: PSUM->SBUF, PE transpose to [feat, graph],
           FC as matmul (+ bias via ones-row matmul), PReLU as one
           scalar_tensor_tensor: out = max(a*x, x)  (valid for a <= 1).
"""
import numpy as np

NUM_GRAPHS = 10000
NODES_PER_GRAPH = 100
FT_IN = 256
NB_CLASSES = 128
N_CORES = 8

G_CORE = NUM_GRAPHS // N_CORES            # 1250 graphs per core
N_CORE = G_CORE * NODES_PER_GRAPH         # 125000 nodes per core
NT_FULL = N_CORE // 128                   # 976 full 128-node tiles
TAIL = N_CORE - NT_FULL * 128             # 72-node tail tile
NT = NT_FULL + 1                          # 977 tiles
GROUPS = (G_CORE + 31) // 32              # 40 groups (39 full + one of 2)
DMA_T = 8                                 # node-tiles per load DMA

_CACHE = {}


def _build_module():
    import concourse.bacc as bacc
    import concourse.mybir as mybir
    from concourse.tile import TileContext

    F32 = mybir.dt.float32
    F32R = mybir.dt.float32r
    F = FT_IN
    C = NB_CLASSES

    patterns = np.zeros((25, 128, 32), dtype=np.float32)
    for r in range(25):
        for i in range(128):
            patterns[r, i, (128 * r + i) // 100] = 1.0
    ident = np.eye(128, dtype=np.float32)
    ones_row = np.ones((1, 128), dtype=np.float32)

    nc = bacc.Bacc(None, target_bir_lowering=False)
    seq = nc.dram_tensor("seq", [N_CORE, F], F32, kind="ExternalInput")
    W = nc.dram_tensor("W", [C, F], F32, kind="ExternalInput")
    b = nc.dram_tensor("b", [1, C], F32, kind="ExternalInput")
    a_in = nc.dram_tensor("a_in", [1, 1], F32, kind="ExternalInput")
    out = nc.dram_tensor("out", [G_CORE, C], F32, kind="ExternalOutput")

    patt_d = nc.inline_tensor(
        patterns.transpose(1, 0, 2).reshape(128, 25 * 32), name="patt")
    id_d = nc.inline_tensor(ident, name="ident")
    ones_d = nc.inline_tensor(ones_row, name="ones_row")

    with TileContext(nc) as tc:
        with (
            tc.tile_pool(name="const", bufs=1) as cpool,
            tc.tile_pool(name="seqp", bufs=8) as seqp,
            tc.tile_pool(name="s2", bufs=3) as s2,
            tc.tile_pool(name="ps1", bufs=4, space="PSUM") as ps1,
            tc.tile_pool(name="ps2", bufs=2, space="PSUM") as ps2,
        ):
            patt = cpool.tile([128, 25 * 32], F32R)
            nc.sync.dma_start(patt[:, :], patt_d[:, :].bitcast(F32R))
            ident_t = cpool.tile([128, 128], F32)
            nc.sync.dma_start(ident_t[:, :], id_d[:, :])
            ones_t = cpool.tile([1, 128], F32)
            nc.sync.dma_start(ones_t[:, :], ones_d[:, :])
            w_sb = cpool.tile([C, F], F32)
            nc.sync.dma_start(w_sb[:, :], W[:, :])
            b_sb = cpool.tile([1, C], F32)
            nc.sync.dma_start(b_sb[:, :], b[:, :])
            a_sb = cpool.tile([1, 1], F32)
            nc.sync.dma_start(a_sb[:, :], a_in[:, :])

            # WT halves [feat_half, class] via PE transpose
            wt_sb = cpool.tile([128, 2 * C], F32)
            for h in range(2):
                ps_w = ps2.tile([128, 128], F32, tag="tp")
                nc.tensor.transpose(
                    ps_w[:, :], w_sb[:, 128 * h:128 * (h + 1)], ident_t[:, :])
                nc.scalar.copy(wt_sb[:, C * h:C * (h + 1)], ps_w[:, :])

            # broadcast prelu_a to a [128, 1] column
            ps_a = ps2.tile([128, 1], F32, tag="tp")
            nc.tensor.matmul(ps_a[:, :], ones_t[:, :], a_sb[:, :],
                             start=True, stop=True)
            a_col = cpool.tile([128, 1], F32)
            nc.scalar.copy(a_col[:, :], ps_a[:, :])

            seq_t = seq[:NT_FULL * 128, :].rearrange("(t p) f -> p t f", p=128)

            # Issue all load DMAs up front; Tile double-buffers via the pool.
            sq_tiles = {}
            for t8 in range(0, NT_FULL, DMA_T):
                sq = seqp.tile([128, DMA_T * F], F32R)
                nc.sync.dma_start(
                    sq[:, :].rearrange("p (t f) -> p t f", t=DMA_T),
                    seq_t[:, t8:t8 + DMA_T, :].bitcast(F32R),
                )
                for k in range(DMA_T):
                    sq_tiles[t8 + k] = (sq, k)
            # tail tile (72 nodes)
            sq_tail = seqp.tile([TAIL, F], F32R, tag="tail")
            nc.sync.dma_start(sq_tail[:, :],
                              seq[NT_FULL * 128:, :].bitcast(F32R))

            for g in range(GROUPS):
                ng = min(32, G_CORE - 32 * g)          # graphs in group
                ntg = (ng * NODES_PER_GRAPH + 127) // 128  # node-tiles
                pooled_ps = ps1.tile([32, F], F32)
                for r in range(ntg):
                    t = 25 * g + r
                    last = r == ntg - 1
                    if t < NT_FULL:
                        sq, k = sq_tiles[t]
                        nc.tensor.matmul(
                            pooled_ps[:, :],
                            patt[:, 32 * r:32 * r + 32],
                            sq[:, F * k:F * (k + 1)],
                            start=(r == 0), stop=last,
                        )
                    else:
                        nc.tensor.matmul(
                            pooled_ps[:, :],
                            patt[:TAIL, 32 * r:32 * r + 32],
                            sq_tail[:, :],
                            start=(r == 0), stop=last,
                        )

                # stage 2 for this group
                pooled_sb = s2.tile([32, F], F32)
                nc.scalar.copy(pooled_sb[:ng, :], pooled_ps[:ng, :])
                pt_sb = s2.tile([128, 64], F32)
                for h in range(2):
                    ps_t = ps2.tile([128, 32], F32, tag="tp")
                    nc.tensor.transpose(
                        ps_t[:, :ng], pooled_sb[:ng, 128 * h:128 * (h + 1)],
                        ident_t[:ng, :ng],
                    )
                    nc.scalar.copy(pt_sb[:, 32 * h:32 * h + ng], ps_t[:, :ng])

                ret_ps = ps2.tile([32, C], F32, tag="ret")
                nc.tensor.matmul(ret_ps[:ng, :], ones_t[:, :ng], b_sb[:, :],
                                 start=True, stop=False)
                for h in range(2):
                    nc.tensor.matmul(
                        ret_ps[:ng, :], pt_sb[:, 32 * h:32 * h + ng],
                        wt_sb[:, C * h:C * (h + 1)],
                        start=False, stop=(h == 1),
                    )
                ret_sb = s2.tile([32, C], F32)
                nc.scalar.copy(ret_sb[:ng, :], ret_ps[:ng, :])
                out_sb = s2.tile([32, C], F32)
                nc.vector.scalar_tensor_tensor(
                    out_sb[:ng, :], ret_sb[:ng, :], a_col[:ng, 0:1],
                    ret_sb[:ng, :],
                    op0=mybir.AluOpType.mult, op1=mybir.AluOpType.max,
                )
                nc.sync.dma_start(out[32 * g:32 * g + ng, :], out_sb[:ng, :])

    nc.finalize()
    return nc


def make_in_maps(inputs):
    seq = np.ascontiguousarray(np.asarray(inputs["seq"], dtype=np.float32))
    W = np.ascontiguousarray(np.asarray(inputs["W"], dtype=np.float32))
    b2 = np.asarray(inputs["b"], dtype=np.float32).reshape(1, NB_CLASSES)
    a2 = np.asarray(inputs["prelu_a"], dtype=np.float32).reshape(1, 1)

    shards = seq.reshape(N_CORES, N_CORE, FT_IN)
    return [
        {"seq": shards[i], "W": W, "b": b2, "a_in": a2}
        for i in range(N_CORES)
    ]


def kernel(seq, graph_len, W, b, prelu_a):
    from concourse.bass_utils import run_bass_kernel_spmd

    if "nc" not in _CACHE:
        _CACHE["nc"] = _build_module()
    nc = _CACHE["nc"]

    in_maps = make_in_maps(
        {"seq": seq, "W": W, "b": b, "prelu_a": prelu_a})
    res = run_bass_kernel_spmd(nc, in_maps, core_ids=list(range(N_CORES)))
    return np.concatenate([r["out"] for r in res.results], axis=0)

